# revision 19
# baseline (speedup 1.0000x reference)
"""Trainium2 Bass kernel for nn_DecentLayer (gnn_message_passing).

The reference gathers 16 of 24 input channels via static position matching,
then runs a 3x3 same-padded conv: [B=16, 16, 256, 256] x [32, 16, 3, 3]
-> [B, 32, 256, 256].

Strategy (v3): row-shift K packing, 2x fewer PE cycles than v2.
  * Data-parallel over batch: 8 cores x 2 images.
  * K = 96 partitions = 6 row-shifts (j) x 16 channels. Partition (j, ch)
    holds the padded rows 32s + 4m' + j (m' = 0..7) of each 32-row strip s,
    so for a 4-row output block at base h (h = 4m'), ALL THREE dh taps of
    output rows h..h+3 read the same per-partition row index m'.
    Data replication is 1.5x (6 j-copies covering stride-4 rows).
  * M = 128 = 4 row-residues (rr) x 32 filters. Stationary w[(j,ch),(rr,f)]
    = W[f, ch, dh=j-rr, dw] for 0 <= j-rr <= 2 (banded), else 0.
  * One matmul per dw in {0,1,2} with N = 512 (2 row-blocks x 256 cols)
    covers all 9 taps for 8 output rows: 3 matmuls / 8 rows -> 37.5%%
    useful-MAC density (vs 18.75%% for the strip-block-diagonal scheme).
  * PSUM: [128, 512] f32 = exactly one bank; 8 banks rotate.
  * Evacuation psum->sbuf (f32->bf16 cast) alternates Vector/Scalar engines
    so neither exceeds ~half the PE time.
  * Input loads on the Sync queue (HWDGE), stores on GpSimd (SWDGE ring).
"""

import numpy as np
import ml_dtypes

import concourse.bass as bass
import concourse.bacc as bacc
import concourse.mybir as mybir
import concourse.tile as tile
from concourse.bass_utils import run_bass_kernel_spmd

# Problem constants (hardcoded per the harness contract).
N_CORES = 8
B = 16
IMGS_PER_CORE = B // N_CORES  # 2
CIN = 16      # conv input channels after gather
COUT = 32     # filters
H = W = 256
SLOTS = 8         # strips per image
HS = H // SLOTS   # 32 output rows per strip
WP = W + 2        # padded row width
NJ = 6            # row-shift copies (j = rr + dh, rr in 0..3, dh in 0..2)
KP = NJ * CIN     # 96 K partitions
NQ = 4            # psum tiles per strip (8 output rows each)
NDW = 3           # matmuls per psum tile (one per dw)


def _common_pairs(ms_in, ns_in, ms_x, ns_x):
    ms_in = np.asarray(ms_in)
    ns_in = np.asarray(ns_in)
    ms_x = np.asarray(ms_x)
    ns_x = np.asarray(ns_x)
    f_ids, x_ids = [], []
    for i_in in range(ms_in.shape[0]):
        hits = np.nonzero((ms_x == ms_in[i_in]) & (ns_x == ns_in[i_in]))[0]
        for i_x in hits:
            f_ids.append(i_in)
            x_ids.append(int(i_x))
    return np.asarray(f_ids), np.asarray(x_ids)


def build_program(n_img=IMGS_PER_CORE):
    f32 = mybir.dt.float32
    bf16 = mybir.dt.bfloat16

    nc = bacc.Bacc("TRN2", target_bir_lowering=False, debug=False)
    # x: per (img, strip): [96 parts, 2 halves, 4 rows, WP] contiguous.
    x_in = nc.dram_tensor("x", [n_img, SLOTS, KP, 2, 4, WP], bf16,
                          kind="ExternalInput")
    w_in = nc.dram_tensor("w", [KP, NDW, 128], bf16, kind="ExternalInput")
    # Permuted output layout: output row h = 32*s + 4*k + rr (k = 2*q + blk).
    # Stored as [b, s, rr, co, k, w] so each per-strip store is a fully
    # contiguous 256 KB block (2 KB per partition); host transposes back.
    y_out = nc.dram_tensor("y", [n_img, SLOTS, 4, COUT, 8, W], bf16,
                           kind="ExternalOutput")

    with tile.TileContext(nc) as tc:
        with (
            tc.tile_pool(name="persist", bufs=1) as persist,
            tc.tile_pool(name="op", bufs=4) as op,
            tc.tile_pool(name="ps", bufs=7, space="PSUM") as psp,
            tc.tile_pool(name="psw", bufs=1, space="PSUM") as pswp,
        ):
            # Weights on the gpsimd queue: its DGE gen runs in parallel with
            # the sync queue's x-load gens, so neither delays the other.
            wt = persist.tile([KP, NDW, 128], bf16, name="wt")
            nc.gpsimd.dma_start(out=wt[:], in_=w_in[:])

            # PE p-state warmup: the PE clock ramps to 2.4 GHz only after
            # ~3 us of continuous execution. Burn that ramp on dummy matmuls
            # over a memset scratch tile while the first loads are in flight,
            # so real matmuls run at full clock from the start.
            scr = persist.tile([KP, 512], bf16, name="scr")
            nc.gpsimd.memset(scr[:], 0.0)
            ps_w = pswp.tile([128, 2 * W], f32, name="warm")
            for _ in range(2):
                nc.tensor.matmul(ps_w[:], scr[:, :128], scr[:, :512],
                                 start=True, stop=True)

            # All x tiles resident (66 KB/partition on 96 partitions).
            # Two tiles per (img, strip): rows m' 0..3 (q=0,1) and 4..7
            # (q=2,3); the very first is split again so the PE can start
            # after a quarter-strip (~100 KB) lands.
            xbufs = {}
            for p in range(n_img):
                for s in range(SLOTS):
                    for hf in range(2):
                        if p == 0 and s == 0 and hf == 0:
                            xa = persist.tile([KP, 2, WP], bf16, name="x000a")
                            xb = persist.tile([KP, 2, WP], bf16, name="x000b")
                            nc.sync.dma_start(out=xa[:],
                                              in_=x_in[0, 0, :, 0, 0:2])
                            nc.sync.dma_start(out=xb[:],
                                              in_=x_in[0, 0, :, 0, 2:4])
                            xbufs[p, s, hf] = (xa, xb)
                        else:
                            xt = persist.tile([KP, 4, WP], bf16,
                                              name=f"x{p}s{s}h{hf}")
                            nc.sync.dma_start(out=xt[:], in_=x_in[p, s, :, hf])
                            xbufs[p, s, hf] = (xt,)

            for p in range(n_img):
                for s in range(SLOTS):
                    last = p == n_img - 1 and s == SLOTS - 1
                    outt = op.tile([128, NQ * 2 * W], bf16, name="ot")
                    if not last:
                        for q in range(NQ):
                            hf, mq = divmod(q, 2)
                            seg = xbufs[p, s, hf]
                            if len(seg) == 2:
                                xv = seg[mq][:, 0:2]
                            else:
                                xv = seg[0][:, 2 * mq : 2 * mq + 2]
                            ps = psp.tile([128, 2 * W], f32, name="acc")
                            for dw in range(NDW):
                                nc.tensor.matmul(
                                    ps[:],
                                    wt[:, dw, :],
                                    xv[:, :, dw : dw + W],
                                    start=(dw == 0),
                                    stop=(dw == NDW - 1),
                                )
                            dst = outt[:, q * 2 * W : (q + 1) * 2 * W]
                            if q % 2 == 0:
                                nc.vector.tensor_copy(dst, ps[:])
                            else:
                                nc.scalar.copy(dst, ps[:])
                        nc.gpsimd.dma_start(out=y_out[p, s], in_=outt[:])
                    else:
                        # Last strip: stores split per-q across two queues so
                        # the final gen isn't serialized behind earlier gens,
                        # and the final cast is split across both engines.
                        for q in range(NQ):
                            hf, mq = divmod(q, 2)
                            xv = xbufs[p, s, hf][0][:, 2 * mq : 2 * mq + 2]
                            ps = psp.tile([128, 2 * W], f32, name="acc")
                            for dw in range(NDW):
                                nc.tensor.matmul(
                                    ps[:],
                                    wt[:, dw, :],
                                    xv[:, :, dw : dw + W],
                                    start=(dw == 0),
                                    stop=(dw == NDW - 1),
                                )
                            dst = outt[:, q * 2 * W : (q + 1) * 2 * W]
                            if q == NQ - 1:
                                nc.vector.tensor_copy(dst[:, :W], ps[:, :W])
                                nc.scalar.copy(dst[:, W:], ps[:, W:])
                                nc.gpsimd.dma_start(
                                    out=y_out[p, s, :, :, 2 * q : 2 * q + 2],
                                    in_=dst,
                                )
                            else:
                                if q % 2 == 0:
                                    nc.vector.tensor_copy(dst, ps[:])
                                else:
                                    nc.scalar.copy(dst, ps[:])
                                if q == 1:
                                    nc.gpsimd.dma_start(
                                        out=y_out[p, s, :, :, 0:4],
                                        in_=outt[:, 0 : 4 * W],
                                    )
                                elif q == 2:
                                    nc.sync.dma_start(
                                        out=y_out[p, s, :, :, 4:6], in_=dst
                                    )

    nc.compile()
    return nc


_NC_CACHE = {}


def _get_program():
    if "v3" not in _NC_CACHE:
        _NC_CACHE["v3"] = build_program()
    return _NC_CACHE["v3"]


def _host_prep(inputs):
    x = np.asarray(inputs["x_data"], dtype=np.float32)
    w = np.asarray(inputs["weights"], dtype=np.float32)
    f_ids, x_ids = _common_pairs(
        inputs["ms_in"], inputs["ns_in"], inputs["ms_x"], inputs["ns_x"]
    )
    assert len(f_ids) == CIN, f"expected {CIN} matched pairs, got {len(f_ids)}"
    xg = x[:, x_ids]                                 # [B, 16, H, W]
    wg = w[:, f_ids]                                 # [COUT, 16, 3, 3]

    bf = ml_dtypes.bfloat16
    pad = np.zeros((B, CIN, H + 2, WP), dtype=bf)
    pad[:, :, 1 : H + 1, 1 : W + 1] = xg.astype(bf)

    # host_x[b, s, 16j+ch, m', c] = pad[b, ch, 32s + 4m' + j, c]
    host = np.empty((B, SLOTS, KP, 8, WP), dtype=bf)
    sm = 32 * np.arange(SLOTS)[:, None] + 4 * np.arange(8)[None, :]  # [s, m']
    for j in range(NJ):
        # pad[:, :, sm+j, :] -> [B, ch, s, m', c] -> [B, s, ch, m', c]
        host[:, :, CIN * j : CIN * (j + 1)] = pad[:, :, sm + j, :].transpose(
            0, 2, 1, 3, 4
        )
    host = host.reshape(B, SLOTS, KP, 2, 4, WP)

    # Stationaries [96, 3, 128]: w[(j,ch), dw, (rr,f)] = wg[f,ch,j-rr,dw]
    w_host = np.zeros((KP, NDW, 128), dtype=np.float32)
    for j in range(NJ):
        for rr in range(4):
            dh = j - rr
            if 0 <= dh <= 2:
                for dw in range(NDW):
                    w_host[CIN * j : CIN * (j + 1), dw,
                           32 * rr : 32 * rr + 32] = wg[:, :, dh, dw].T
    w_host = w_host.astype(bf)
    return host, w_host


def _run(inputs, trace=False):
    xh, w_host = _host_prep(inputs)
    nc = _get_program()
    in_maps = [
        {"x": xh[IMGS_PER_CORE * k : IMGS_PER_CORE * (k + 1)], "w": w_host}
        for k in range(N_CORES)
    ]
    res = run_bass_kernel_spmd(nc, in_maps, list(range(N_CORES)), trace=trace)
    # y stored as [n_img, s, rr, co, k, w]; h = 32*s + 4*k + rr
    outs = []
    for r in res.results:
        yp = np.asarray(r["y"]).astype(np.float32)
        outs.append(
            yp.transpose(0, 3, 1, 4, 2, 5).reshape(IMGS_PER_CORE, COUT, H, W)
        )
    out = np.concatenate(outs, axis=0)
    return out, res


def kernel(**inputs):
    out, _ = _run(inputs, trace=False)
    return out
